# revision 18
# baseline (speedup 1.0000x reference)
"""Trainium2 Bass kernel for nn_DIMPA (3-hop dual-graph COO SpMM).

Strategy (8 NeuronCores, SPMD single program):
  - Destination nodes sharded across cores (12500 rows/core).
  - Host pre-sorts each core's edges by (dest-tile-of-128, src-quartile,
    dest-half), builds int16 gather indices (quartile-relative so they fit
    int16), bf16 edge values and bf16 local-dest ids, laid out per
    128-edge chunk.
  - Device, per dest tile: SWDGE dma_gather of bf16 source rows from HBM,
    DVE builds a one-hot "segment matrix" (iota == dst_local) and scales
    gathered rows by edge values, PE computes onehot.T @ feats which IS
    the segment-sum (scatter-add) into PSUM, accumulated over chunks.
  - feat accumulators (w[h] * curr_h) live in SBUF for the whole kernel.
  - Hop-1 source: each core receives only ITS OWN x shard (bf16); an
    AllGather rebuilds the full N x D source in device DRAM. Same for
    hops 2/3 from the previous hop's results.
  - All host->device payloads are per-core shards / compact bf16
    metadata (~6 MB/core); the output returns as bf16 and is widened to
    f32 on the host. This keeps the axon transfer small.
"""

import math
from contextlib import ExitStack, nullcontext

import numpy as np

import concourse.bass as bass
import concourse.bacc as bacc
import concourse.tile as tile
from concourse import library_config, mybir
from concourse.bass_utils import run_bass_kernel_spmd

F32 = mybir.dt.float32
BF16 = mybir.dt.bfloat16
I16 = mybir.dt.int16
I32 = mybir.dt.int32


class Cfg:
    def __init__(self, N=100000, E=1200000, D=64, HOP=3, CORES=8, NQ=4,
                 debug=False, cnt_reg=True, **_ignored):
        assert N % CORES == 0 and N % NQ == 0
        self.N, self.E, self.D, self.HOP, self.CORES, self.NQ = N, E, D, HOP, CORES, NQ
        self.NPC = N // CORES              # nodes per core
        self.TILES = math.ceil(self.NPC / 128)
        self.TAIL = self.NPC - (self.TILES - 1) * 128
        self.QROWS = N // NQ               # rows per source quartile
        assert self.QROWS <= 32767, "gather idx must fit int16"
        self.debug = debug
        self.cnt_reg = cnt_reg             # runtime valid-count per gather
        self.mock_cc = False               # timing-sim only: no collectives
        self.repeat = 1                    # timing only: loop the whole body
        self.diag = None                   # 'gathers_only' | 'no_gathers'
        self.scratch = 32768               # SWDGE descriptor-ring bytes
        self.nqueues = 4                   # SWDGE queues for gathers


def _preprocess_graph(cfg, rows, cols, vals):
    """Per-core edge layout. Edges keyed by (dest-tile, src-quartile,
    dest-half): each 128-edge chunk targets one 64-row half of the dest tile
    so the one-hot segment matrix is only 64 wide and LDWEIGHTS is 64 cols.
    One gather call per (tile, quartile) covers its h0+h1 chunks
    contiguously."""
    import ml_dtypes
    NQ, T = cfg.NQ, cfg.TILES
    NCELL = T * NQ * 2                     # (t, q, h) cells
    rows = np.asarray(rows); cols = np.asarray(cols); vals = np.asarray(vals)
    core = rows // cfg.NPC
    per_core = []
    for c in range(cfg.CORES):
        sel = core == c
        r = rows[sel] - c * cfg.NPC
        s = cols[sel]
        v = vals[sel]
        t = r // 128
        dl = r % 128
        h = dl // 64
        q = s // cfg.QROWS
        i16 = (s % cfg.QROWS).astype(np.int16)
        key = (t * NQ + q) * 2 + h
        order = np.argsort(key, kind="stable")
        per_core.append((key[order], i16[order], v[order],
                         (dl[order] % 64)))
    counts = [np.bincount(k, minlength=NCELL) for k, _, _, _ in per_core]

    # schedule: chunks per (t, q, h), same for all cores
    mx = np.max(np.stack(counts, 0), axis=0).reshape(T, NQ, 2)
    kq = -(-mx // 128)                     # ceil
    for t in range(T):                     # every psum half needs >=1 chunk
        for h in range(2):
            if kq[t, :, h].sum() == 0:
                kq[t, 0, h] = 1

    kt = kq.sum(axis=(1, 2))                          # chunks per tile
    tb = np.concatenate([[0], np.cumsum(kt)])         # tile chunk base
    TC = int(tb[-1])
    # chunk offset of (q, h) within tile: q-major, then h
    qoff = np.zeros((T, NQ, 2), np.int64)
    halves = []                                       # per tile: half of chunk
    for t in range(T):
        off = 0
        hs = []
        for q in range(NQ):
            for h in range(2):
                qoff[t, q, h] = off
                off += kq[t, q, h]
                hs += [h] * int(kq[t, q, h])
        halves.append(hs)
    # idx columns per (t,q) call: 8 cols per chunk
    kq_call = kq.sum(axis=2)                          # [T, NQ]
    ib = np.concatenate([[0], np.cumsum(kq_call.reshape(-1) * 8)])
    IC = int(ib[-1])

    call_of = -np.ones(T * NQ, np.int64)
    ncalls = 0
    for t in range(T):
        for q in range(NQ):
            if kq_call[t, q] > 0:
                call_of[t * NQ + q] = ncalls
                ncalls += 1

    core_arrays = []
    for (key, i16, v, dl64), cnts in zip(per_core, counts):
        val_dev = np.zeros((128, TC), ml_dtypes.bfloat16)
        dst_dev = np.zeros((128, TC), ml_dtypes.bfloat16)
        idx_dev = np.zeros((16, IC), np.int16)
        cnts2 = cnts.reshape(T, NQ, 2)
        if len(key):
            gstart = np.concatenate([[0], np.cumsum(cnts)])[:-1]
            j = np.arange(len(key)) - gstart[key]     # pos within cell
            tt = key // (NQ * 2)
            qq = (key // 2) % NQ
            hh = key % 2
            gchunk = tb[tt] + qoff[tt, qq, hh] + j // 128
            lane = j % 128
            val_dev[lane, gchunk] = v
            dst_dev[lane, gchunk] = dl64
            # idx position within the (t,q) call: h0 block then h1 block
            jc = j + (hh * kq[tt, qq, 0] * 128)
            col = ib[tt * NQ + qq] + jc // 16
            part = (jc % 16).astype(np.int64)
            idx_dev[part, col] = i16
        cnt_dev = np.zeros((1, max(ncalls, 1)), np.int32)
        if cfg.cnt_reg:
            # mark the h1 tail invalid (-1); h0 pads stay idx 0 (mid-call
            # negatives are not allowed)
            for t in range(T):
                for q in range(NQ):
                    ci = call_of[t * NQ + q]
                    if ci < 0:
                        continue
                    k0 = int(kq[t, q, 0])
                    k1 = int(kq[t, q, 1])
                    n1 = int(cnts2[t, q, 1])
                    base = k0 * 128
                    if k1 > 0 and n1 < k1 * 128:
                        if n1 == 0:
                            n1 = 1        # keep a valid tail descriptor
                        jj = np.arange(base + n1, base + k1 * 128)
                        colp = ib[t * NQ + q] + jj // 16
                        pp = jj % 16
                        idx_dev[pp, colp] = -1
                    cnt_dev[0, ci] = base + n1 if k1 > 0 else base
        core_arrays.append({"idx": idx_dev, "val": val_dev, "dst": dst_dev,
                            "cnt": cnt_dev})
    meta = {"kq": kq, "kt": kt, "tb": tb, "TC": TC, "qoff": qoff,
            "kq_call": kq_call, "halves": halves,
            "ib": ib.reshape(-1), "IC": IC, "call_of": call_of,
            "ncalls": max(ncalls, 1)}
    return meta, core_arrays


def build_program(cfg, meta_s, meta_t):
    nc = bacc.Bacc("TRN2", target_bir_lowering=False, debug=cfg.debug,
                   num_devices=cfg.CORES,
                   dynamic_dma_scratch_size=cfg.scratch,
                   num_swdge_queues=cfg.nqueues)
    N, D, HOP, TILES, TAIL = cfg.N, cfg.D, cfg.HOP, cfg.TILES, cfg.TAIL
    NPC, NQ, QROWS = cfg.NPC, cfg.NQ, cfg.QROWS
    graphs = ("s", "t")
    metas = {"s": meta_s, "t": meta_t}

    # ---- I/O (all per-core shards / compact metadata) ----
    xsh = {g: nc.dram_tensor(f"xsh_{g}", [TILES * 128, D], BF16,
                             kind="ExternalInput") for g in graphs}
    idx_d = {g: nc.dram_tensor(f"idx_{g}", [16, metas[g]["IC"]], I16,
                               kind="ExternalInput") for g in graphs}
    val_d = {g: nc.dram_tensor(f"val_{g}", [128, metas[g]["TC"]], BF16,
                               kind="ExternalInput") for g in graphs}
    dst_d = {g: nc.dram_tensor(f"dst_{g}", [128, metas[g]["TC"]], BF16,
                               kind="ExternalInput") for g in graphs}
    iota_d = nc.dram_tensor("iotab", [128, 64], F32, kind="ExternalInput")
    wb_d = {g: nc.dram_tensor(f"wb_{g}", [128, HOP + 1], F32,
                              kind="ExternalInput") for g in graphs}
    cnt_d = {g: nc.dram_tensor(f"cnt_{g}", [1, metas[g]["ncalls"]], I32,
                               kind="ExternalInput") for g in graphs} \
        if cfg.cnt_reg else None
    out_d = nc.dram_tensor("out", [NPC, 2 * D], BF16, kind="ExternalOutput")

    # ---- internal DRAM: hop sources (full N rows, assembled by AllGather).
    # f32 rows are 256 B — the SWDGE gather granularity — so no pad cols.
    cur_nxt = {g: {h: nc.dram_tensor(f"curnxt_{g}{h}", [TILES * 128, D],
                                     F32)
                   for h in range(0, HOP)} for g in graphs}
    cur_ful = {g: {h: nc.dram_tensor(f"curful_{g}{h}", [N, D], F32,
                                     addr_space="Shared")
                   for h in range(0, HOP)} for g in graphs}

    ktmax = max(int(metas[g]["kt"].max()) for g in graphs)

    with tile.TileContext(nc) as tc, ExitStack() as ctx:
        meta_p = ctx.enter_context(tc.tile_pool(name="meta", bufs=1))
        feat_p = ctx.enter_context(tc.tile_pool(name="feat", bufs=1))
        g_p = ctx.enter_context(tc.tile_pool(name="gather", bufs=3))
        oh_p = ctx.enter_context(tc.tile_pool(name="onehot", bufs=3))
        ps_p = ctx.enter_context(tc.tile_pool(name="psum", bufs=4,
                                              space="PSUM"))
        st_p = ctx.enter_context(tc.tile_pool(name="stage", bufs=3))
        once_p = ctx.enter_context(tc.tile_pool(name="once", bufs=1))

        nc.gpsimd.load_library(library_config.mlp)

        iota_b = meta_p.tile([128, 64], F32)
        nc.sync.dma_start(iota_b[:], iota_d[:, :])

        cnt_regs = None
        gt_bufs = None
        if cfg.cnt_reg:
            cnt_regs = [ctx.enter_context(nc.gpsimd.register(f"cntreg{i}"))
                        for i in range(4)]
            # Fixed gather buffers (manual round-robin): skipped (padded)
            # gather rows must read as finite so that 0*val stays 0, so we
            # zero each buffer exactly once up front.
            gt_bufs = [meta_p.tile([128, ktmax, D], F32, name=f"gtbuf{i}")
                       for i in range(3)]
            for b in gt_bufs:
                nc.vector.memset(b[:], 0.0)

        idx_t, val_t, dst_t, wb_t, feat, cnt_t = {}, {}, {}, {}, {}, {}
        for g in graphs:
            # idx arrives as [16, IC]; the SWDGE consumes it wrapped in 16
            # partitions replicated across the 8 gpsimd cores' partition
            # groups -> replicate on-device with 8 cheap DMAs.
            idx_t[g] = meta_p.tile([128, metas[g]["IC"]], I16,
                                   tag=f"idx{g}", name=f"idx_t_{g}")
            for grp in range(8):
                nc.sync.dma_start(idx_t[g][16 * grp:16 * (grp + 1), :],
                                  idx_d[g][:, :])
            # val/dst ship as bf16 and widen to f32 on device (DVE input
            # dtypes must match the f32 gather rows / f32 iota).
            vb = once_p.tile([128, metas[g]["TC"]], BF16, tag="vdb")
            nc.sync.dma_start(vb[:], val_d[g][:, :])
            val_t[g] = meta_p.tile([128, metas[g]["TC"]], F32,
                                   tag=f"val{g}", name=f"val_t_{g}")
            nc.vector.tensor_copy(val_t[g][:], vb[:])
            db = once_p.tile([128, metas[g]["TC"]], BF16, tag="vdb")
            nc.sync.dma_start(db[:], dst_d[g][:, :])
            dst_t[g] = meta_p.tile([128, metas[g]["TC"]], F32,
                                   tag=f"dst{g}", name=f"dst_t_{g}")
            nc.vector.tensor_copy(dst_t[g][:], db[:])
            wb_t[g] = meta_p.tile([128, HOP + 1], F32, tag=f"wb{g}",
                                  name=f"wb_t_{g}")
            nc.sync.dma_start(wb_t[g][:], wb_d[g][:, :])
            if cfg.cnt_reg:
                cnt_t[g] = meta_p.tile([1, metas[g]["ncalls"]], I32,
                                       tag=f"cnt{g}", name=f"cnt_t_{g}")
                nc.sync.dma_start(cnt_t[g][:], cnt_d[g][:, :])
            # feat init: feat = w[0] * x_own (bf16 shard -> f32 accumulator).
            # The unscaled f32 x shard is also written back to DRAM as the
            # hop-1 AllGather payload (gather rows must be 256 B = f32*D).
            xsh_t = once_p.tile([128, TILES, D], BF16, tag="xsh",
                                name=f"xsh_t_{g}")
            nc.sync.dma_start(
                xsh_t[:],
                xsh[g].ap().rearrange("(t p) d -> p t d", p=128))
            feat[g] = feat_p.tile([128, TILES, D], F32, tag=f"feat{g}",
                                  name=f"feat_{g}")
            nc.vector.tensor_copy(feat[g][:].rearrange("p t d -> p (t d)"),
                                  xsh_t[:].rearrange("p t d -> p (t d)"))
            nc.sync.dma_start(
                cur_nxt[g][0].ap().rearrange("(t p) d -> p t d", p=128),
                feat[g][:])
            nc.vector.tensor_scalar_mul(
                feat[g][:].rearrange("p t d -> p (t d)"),
                feat[g][:].rearrange("p t d -> p (t d)"),
                wb_t[g][:, 0:1])
            # hop-1 gather source: AllGather the x shards into full N x D
            if cfg.mock_cc:
                for r in range(cfg.CORES):
                    nc.sync.dma_start(
                        cur_ful[g][0][r * NPC:(r + 1) * NPC, :],
                        cur_nxt[g][0][0:NPC, :])
            else:
                nc.gpsimd.collective_compute(
                    "AllGather", mybir.AluOpType.bypass,
                    replica_groups=[list(range(cfg.CORES))],
                    ins=[cur_nxt[g][0][0:NPC, :].opt()],
                    outs=[cur_ful[g][0].ap().opt()])

        tile_rr = 0
        rep_ctx = (tc.For_i(0, cfg.repeat, 1) if cfg.repeat > 1
                   else nullcontext())
        ctx.enter_context(rep_ctx)
        for h in range(1, HOP + 1):
            for g in graphs:
                m = metas[g]
                src = cur_ful[g][h - 1]
                for t in range(TILES):
                    kt = int(m["kt"][t])
                    halves = m["halves"][t]
                    if cfg.cnt_reg:
                        gt = gt_bufs[tile_rr % 3][:, :kt, :]
                        tile_rr += 1
                    else:
                        gt = g_p.tile([128, kt, D], F32, tag="gt")
                    for q in range(NQ):
                        kq = int(m["kq_call"][t, q])
                        if kq == 0:
                            continue
                        qo = int(m["qoff"][t, q, 0])
                        ibase = int(m["ib"][t * NQ + q])
                        if cfg.cnt_reg:
                            ci = int(m["call_of"][t * NQ + q])
                            reg = cnt_regs[ci % 4]
                            nc.gpsimd.reg_load(reg,
                                               cnt_t[g][0:1, ci:ci + 1])
                            nreg = reg
                        else:
                            nreg = kq * 128
                        if cfg.diag != "no_gathers":
                            nc.gpsimd.dma_gather(
                                gt[:, qo:qo + kq, :],
                                src[q * QROWS:(q + 1) * QROWS, :],
                                idx_t[g][:, ibase:ibase + kq * 8],
                                kq * 128, nreg, D,
                                queue_num=(t * NQ + q) % cfg.nqueues)
                    tb = int(m["tb"][t])
                    if cfg.diag == "gathers_only":
                        continue
                    oh = oh_p.tile([128, kt, 64], BF16, tag="oh")
                    nc.vector.tensor_tensor(
                        oh[:],
                        iota_b[:, 0:64].unsqueeze(1)
                            .broadcast_to([128, kt, 64]),
                        dst_t[g][:, tb:tb + kt].unsqueeze(2)
                            .broadcast_to([128, kt, 64]),
                        mybir.AluOpType.is_equal)
                    rhs = oh_p.tile([128, kt, D], BF16, tag="gtb",
                                    name="gtb")
                    nc.vector.tensor_tensor(
                        rhs[:],
                        gt[:],
                        val_t[g][:, tb:tb + kt].unsqueeze(2)
                            .broadcast_to([128, kt, D]),
                        mybir.AluOpType.mult)
                    ps = ps_p.tile([128, D], F32)
                    first = {0: True, 1: True}
                    last_of = {}
                    for c, hc in enumerate(halves):
                        last_of[hc] = c
                    for c, hc in enumerate(halves):
                        nc.tensor.matmul(
                            ps[hc * 64:(hc + 1) * 64, :],
                            oh[:, c, :], rhs[:, c, :],
                            start=first[hc], stop=(c == last_of[hc]),
                            tile_position=(0, hc * 64),
                            skip_group_check=True)
                        first[hc] = False
                    nc.vector.scalar_tensor_tensor(
                        feat[g][:, t, :], ps[:], wb_t[g][:, h:h + 1],
                        feat[g][:, t, :],
                        mybir.AluOpType.mult, mybir.AluOpType.add)
                    if h < HOP:
                        rows = TAIL if t == TILES - 1 else 128
                        st = st_p.tile([128, D], F32)
                        nc.scalar.copy(st[:], ps[:])
                        nc.sync.dma_start(
                            cur_nxt[g][h][t * 128:t * 128 + rows, :],
                            st[:rows, :])
                if h < HOP:
                    if cfg.mock_cc:
                        # timing-model stand-in for the AllGather: move the
                        # same number of received bytes through the DMA path
                        for r in range(cfg.CORES):
                            nc.sync.dma_start(
                                cur_ful[g][h][r * NPC:(r + 1) * NPC, :],
                                cur_nxt[g][h][0:NPC, :])
                    else:
                        nc.gpsimd.collective_compute(
                            "AllGather", mybir.AluOpType.bypass,
                            replica_groups=[list(range(cfg.CORES))],
                            ins=[cur_nxt[g][h][0:NPC, :].opt()],
                            outs=[cur_ful[g][h].ap().opt()])

        # ---- write output: out[:, 0:D] = feat_s, out[:, D:2D] = feat_t ----
        for g, co in (("s", 0), ("t", D)):
            ob = once_p.tile([128, TILES, D], BF16, tag="ob", name=f"ob_{g}")
            nc.vector.tensor_copy(ob[:].rearrange("p t d -> p (t d)"),
                                  feat[g][:].rearrange("p t d -> p (t d)"))
            full_t = TILES - 1
            if full_t > 0:
                nc.sync.dma_start(
                    out_d[0:full_t * 128, co:co + D].rearrange(
                        "(t p) d -> p t d", p=128),
                    ob[:, 0:full_t, :])
            nc.sync.dma_start(
                out_d[full_t * 128:NPC, co:co + D],
                ob[0:TAIL, full_t, :])

    return nc


def _make_in_maps(cfg, inputs, meta_s, arrs_s, meta_t, arrs_t):
    import ml_dtypes
    x_s = np.asarray(inputs["x_s"], np.float32)
    x_t = np.asarray(inputs["x_t"], np.float32)
    w_s = np.asarray(inputs["w_s"], np.float32)
    w_t = np.asarray(inputs["w_t"], np.float32)
    wb_s = np.tile(w_s.reshape(1, -1), (128, 1)).astype(np.float32)
    wb_t = np.tile(w_t.reshape(1, -1), (128, 1)).astype(np.float32)
    iotab = np.tile(np.arange(64, dtype=np.float32), (128, 1))
    in_maps = []
    for c in range(cfg.CORES):
        xo_s = np.zeros((cfg.TILES * 128, cfg.D), ml_dtypes.bfloat16)
        xo_s[:cfg.NPC] = x_s[c * cfg.NPC:(c + 1) * cfg.NPC]
        xo_t = np.zeros((cfg.TILES * 128, cfg.D), ml_dtypes.bfloat16)
        xo_t[:cfg.NPC] = x_t[c * cfg.NPC:(c + 1) * cfg.NPC]
        im = {
            "xsh_s": xo_s, "xsh_t": xo_t,
            "idx_s": arrs_s[c]["idx"], "idx_t": arrs_t[c]["idx"],
            "val_s": arrs_s[c]["val"], "val_t": arrs_t[c]["val"],
            "dst_s": arrs_s[c]["dst"], "dst_t": arrs_t[c]["dst"],
            "wb_s": wb_s, "wb_t": wb_t,
            "iotab": iotab,
        }
        if cfg.cnt_reg:
            im["cnt_s"] = arrs_s[c]["cnt"]
            im["cnt_t"] = arrs_t[c]["cnt"]
        in_maps.append(im)
    return in_maps


def prepare(cfg, inputs):
    meta_s, arrs_s = _preprocess_graph(
        cfg, inputs["A_rows"], inputs["A_cols"], inputs["A_vals"])
    meta_t, arrs_t = _preprocess_graph(
        cfg, inputs["At_rows"], inputs["At_cols"], inputs["At_vals"])
    nc = build_program(cfg, meta_s, meta_t)
    nc.compile()
    in_maps = _make_in_maps(cfg, inputs, meta_s, arrs_s, meta_t, arrs_t)
    return nc, in_maps


def kernel(**inputs) -> np.ndarray:
    cfg = Cfg()
    nc, in_maps = prepare(cfg, inputs)
    res = run_bass_kernel_spmd(nc, in_maps, list(range(cfg.CORES)))
    return np.concatenate(
        [res.results[c]["out"].astype(np.float32)
         for c in range(cfg.CORES)], axis=0)


# revision 21
# speedup vs baseline: 2.5697x; 2.5697x over previous
"""Trainium2 Bass kernel for nn_DIMPA (3-hop dual-graph COO SpMM).

Strategy (8 NeuronCores, SPMD single program):
  - Destination nodes sharded across cores (12500 rows/core, 98 tiles of
    128 dest rows each).
  - Host buckets each core's edges by (dest-tile, src-quartile), pads
    every bucket to a uniform K 128-edge chunks, and lays out int16
    gather indices (quartile-relative so they fit int16), bf16 edge
    values and bf16 local-dest ids per chunk. Pad slots keep idx=0 and
    val=0 so they contribute nothing.
  - Device, per dest tile (a hardware For_i loop over tiles): SWDGE
    dma_gather of f32 source rows (256 B each) from HBM, DVE builds a
    one-hot "segment matrix" (iota == dst_local) and scales gathered
    rows by edge values (both cast to bf16), PE computes onehot.T @ rows
    which IS the segment-sum (scatter-add) into PSUM, accumulated over
    the tile's NQ*K chunks.
  - feat accumulators (w[h] * curr_h) live in SBUF for the whole kernel.
  - Hop sources: each core receives only ITS OWN x shard (bf16); an
    AllGather rebuilds the full N x D f32 source in device DRAM before
    each hop.
  - All host->device payloads are per-core shards / compact bf16 or i16
    metadata (~6 MB/core); the output returns as bf16 and is widened to
    f32 on the host. This keeps the axon transfer small, and the
    hardware loop keeps program build + BIR compile time small.
"""

import math
from contextlib import ExitStack, nullcontext

import numpy as np

import concourse.bass as bass
import concourse.bacc as bacc
import concourse.tile as tile
from concourse import library_config, mybir
from concourse.bass import ds
from concourse.bass_utils import run_bass_kernel_spmd

F32 = mybir.dt.float32
BF16 = mybir.dt.bfloat16
I16 = mybir.dt.int16
I32 = mybir.dt.int32


class Cfg:
    def __init__(self, N=100000, E=1200000, D=64, HOP=3, CORES=8, NQ=4,
                 debug=False, **_ignored):
        assert N % CORES == 0 and N % NQ == 0
        self.N, self.E, self.D, self.HOP, self.CORES, self.NQ = N, E, D, HOP, CORES, NQ
        self.NPC = N // CORES              # nodes per core
        self.TILES = math.ceil(self.NPC / 128)
        self.TAIL = self.NPC - (self.TILES - 1) * 128
        self.QROWS = N // NQ               # rows per source quartile
        assert self.QROWS <= 32767, "gather idx must fit int16"
        self.debug = debug
        self.mock_cc = False               # timing-sim only: no collectives
        self.diag = None                   # 'gathers_only' | 'no_gathers'
        self.scratch = 32768               # SWDGE descriptor-ring bytes
        self.nqueues = 4                   # SWDGE queues for gathers
        self.unroll = 2                    # tiles per hw-loop iteration


def _preprocess_graph(cfg, rows, cols, vals):
    """Vectorized per-core edge layout with a uniform schedule.

    Edges bucketed by (core, dest-tile, src-quartile); every bucket padded
    to K 128-edge chunks where K = ceil(max bucket size / 128) across all
    cores. Pad slots keep idx 0 / val 0. Returns (K, per-core arrays)."""
    import ml_dtypes
    NQ, T, C = cfg.NQ, cfg.TILES, cfg.CORES
    rows = np.asarray(rows); cols = np.asarray(cols); vals = np.asarray(vals)
    core = rows // cfg.NPC
    r = rows - core * cfg.NPC
    t = r // 128
    dl = (r % 128).astype(np.float32)
    q = cols // cfg.QROWS
    i16 = (cols % cfg.QROWS).astype(np.int16)
    cell = (core * T + t) * NQ + q
    counts = np.bincount(cell, minlength=C * T * NQ)
    K = max(1, -(-int(counts.max()) // 128))
    KT = NQ * K
    TC = T * KT                            # chunks per core
    ICT = KT * 8                           # idx cols per tile
    IC = T * ICT                           # idx cols per core

    order = np.argsort(cell, kind="stable")
    cell_s = cell[order]
    starts = np.concatenate([[0], np.cumsum(counts)])[:-1]
    j = np.arange(len(cell_s)) - starts[cell_s]
    core_s = cell_s // (T * NQ)
    loc = cell_s - core_s * (T * NQ)       # t*NQ + q within core
    gchunk = loc * K + j // 128
    lane = j % 128
    colc = loc * (K * 8) + j // 16
    part = j % 16

    val_dev = np.zeros((C, 128, TC), ml_dtypes.bfloat16)
    dst_dev = np.zeros((C, 128, TC), ml_dtypes.bfloat16)
    idx_dev = np.zeros((C, 16, IC), np.int16)
    val_dev[core_s, lane, gchunk] = vals[order]
    dst_dev[core_s, lane, gchunk] = dl[order]
    idx_dev[core_s, part, colc] = i16[order]
    core_arrays = [{"idx": idx_dev[c], "val": val_dev[c], "dst": dst_dev[c]}
                   for c in range(C)]
    return K, core_arrays


def build_program(cfg, K_s, K_t):
    nc = bacc.Bacc("TRN2", target_bir_lowering=False, debug=cfg.debug,
                   num_devices=cfg.CORES,
                   dynamic_dma_scratch_size=cfg.scratch,
                   num_swdge_queues=cfg.nqueues)
    N, D, HOP, TILES, TAIL = cfg.N, cfg.D, cfg.HOP, cfg.TILES, cfg.TAIL
    NPC, NQ, QROWS, U = cfg.NPC, cfg.NQ, cfg.QROWS, cfg.unroll
    graphs = ("s", "t")
    Ks = {"s": K_s, "t": K_t}

    # ---- I/O (all per-core shards / compact metadata) ----
    xsh = {g: nc.dram_tensor(f"xsh_{g}", [TILES * 128, D], BF16,
                             kind="ExternalInput") for g in graphs}
    idx_d = {g: nc.dram_tensor(f"idx_{g}", [16, TILES * NQ * Ks[g] * 8],
                               I16, kind="ExternalInput") for g in graphs}
    val_d = {g: nc.dram_tensor(f"val_{g}", [128, TILES * NQ * Ks[g]], BF16,
                               kind="ExternalInput") for g in graphs}
    dst_d = {g: nc.dram_tensor(f"dst_{g}", [128, TILES * NQ * Ks[g]], BF16,
                               kind="ExternalInput") for g in graphs}
    iota_d = nc.dram_tensor("iotab", [128, 128], F32, kind="ExternalInput")
    wb_d = {g: nc.dram_tensor(f"wb_{g}", [128, HOP + 1], F32,
                              kind="ExternalInput") for g in graphs}
    out_d = nc.dram_tensor("out", [NPC, 2 * D], BF16, kind="ExternalOutput")

    # ---- internal DRAM: hop sources (full N rows, assembled by AllGather).
    # f32 rows are 256 B — the SWDGE gather granularity — so no pad cols.
    cur_nxt = {g: {h: nc.dram_tensor(f"curnxt_{g}{h}", [TILES * 128, D],
                                     F32)
                   for h in range(0, HOP)} for g in graphs}
    cur_ful = {g: {h: nc.dram_tensor(f"curful_{g}{h}", [N, D], F32,
                                     addr_space="Shared")
                   for h in range(0, HOP)} for g in graphs}

    with tile.TileContext(nc) as tc, ExitStack() as ctx:
        meta_p = ctx.enter_context(tc.tile_pool(name="meta", bufs=1))
        feat_p = ctx.enter_context(tc.tile_pool(name="feat", bufs=1))
        g_p = ctx.enter_context(tc.tile_pool(name="gather", bufs=3))
        oh_p = ctx.enter_context(tc.tile_pool(name="onehot", bufs=3))
        ps_p = ctx.enter_context(tc.tile_pool(name="psum", bufs=4,
                                              space="PSUM"))
        st_p = ctx.enter_context(tc.tile_pool(name="stage", bufs=3))
        once_p = ctx.enter_context(tc.tile_pool(name="once", bufs=1))

        nc.gpsimd.load_library(library_config.mlp)

        iota_b = meta_p.tile([128, 128], F32)
        nc.sync.dma_start(iota_b[:], iota_d[:, :])

        idx_t, val_t, dst_t, wb_t, feat = {}, {}, {}, {}, {}
        for g in graphs:
            TCg = TILES * NQ * Ks[g]
            # idx arrives as [16, IC]; the SWDGE consumes it wrapped in 16
            # partitions replicated across the 8 gpsimd cores' partition
            # groups -> replicate on-device with 8 cheap DMAs.
            idx_t[g] = meta_p.tile([128, TCg * 8], I16,
                                   tag=f"idx{g}", name=f"idx_t_{g}")
            for grp in range(8):
                nc.sync.dma_start(idx_t[g][16 * grp:16 * (grp + 1), :],
                                  idx_d[g][:, :])
            # val/dst ship as bf16 and widen to f32 on device (DVE input
            # dtypes must match the f32 gather rows / f32 iota).
            vb = once_p.tile([128, TCg], BF16, tag="vdb")
            nc.sync.dma_start(vb[:], val_d[g][:, :])
            val_t[g] = meta_p.tile([128, TCg], F32,
                                   tag=f"val{g}", name=f"val_t_{g}")
            nc.vector.tensor_copy(val_t[g][:], vb[:])
            db = once_p.tile([128, TCg], BF16, tag="vdb")
            nc.sync.dma_start(db[:], dst_d[g][:, :])
            dst_t[g] = meta_p.tile([128, TCg], F32,
                                   tag=f"dst{g}", name=f"dst_t_{g}")
            nc.vector.tensor_copy(dst_t[g][:], db[:])
            wb_t[g] = meta_p.tile([128, HOP + 1], F32, tag=f"wb{g}",
                                  name=f"wb_t_{g}")
            nc.sync.dma_start(wb_t[g][:], wb_d[g][:, :])
            # feat init: feat = w[0] * x_own (bf16 shard -> f32 accumulator).
            # The unscaled f32 x shard is also written back to DRAM as the
            # hop-1 AllGather payload (gather rows must be 256 B = f32*D).
            xsh_t = once_p.tile([128, TILES, D], BF16, tag="xsh",
                                name=f"xsh_t_{g}")
            nc.sync.dma_start(
                xsh_t[:],
                xsh[g].ap().rearrange("(t p) d -> p t d", p=128))
            feat[g] = feat_p.tile([128, TILES, D], F32, tag=f"feat{g}",
                                  name=f"feat_{g}")
            nc.vector.tensor_copy(feat[g][:].rearrange("p t d -> p (t d)"),
                                  xsh_t[:].rearrange("p t d -> p (t d)"))
            nc.sync.dma_start(
                cur_nxt[g][0].ap().rearrange("(t p) d -> p t d", p=128),
                feat[g][:])
            nc.vector.tensor_scalar_mul(
                feat[g][:].rearrange("p t d -> p (t d)"),
                feat[g][:].rearrange("p t d -> p (t d)"),
                wb_t[g][:, 0:1])

        def spread(h, g):
            if cfg.mock_cc:
                # timing-model stand-in for the AllGather: move the same
                # number of received bytes through the DMA path
                for r in range(cfg.CORES):
                    nc.sync.dma_start(
                        cur_ful[g][h][r * NPC:(r + 1) * NPC, :],
                        cur_nxt[g][h][0:NPC, :])
            else:
                nc.gpsimd.collective_compute(
                    "AllGather", mybir.AluOpType.bypass,
                    replica_groups=[list(range(cfg.CORES))],
                    ins=[cur_nxt[g][h][0:NPC, :].opt()],
                    outs=[cur_ful[g][h].ap().opt()])

        for g in graphs:
            spread(0, g)

        for h in range(1, HOP + 1):
            for g in graphs:
                K = Ks[g]
                KT = NQ * K
                src = cur_ful[g][h - 1]
                feat2d = feat[g][:].rearrange("p t d -> p (t d)")
                with tc.For_i(0, TILES, U) as iv:
                    for u in range(U):
                        te = iv + u
                        gt = g_p.tile([128, KT, D], F32, tag="gt")
                        if cfg.diag != "no_gathers":
                            for q in range(NQ):
                                nc.gpsimd.dma_gather(
                                    gt[:, q * K:(q + 1) * K, :],
                                    src[q * QROWS:(q + 1) * QROWS, :],
                                    idx_t[g][:, ds(te * (KT * 8)
                                                   + q * (K * 8), K * 8)],
                                    K * 128, K * 128, D,
                                    queue_num=q % cfg.nqueues)
                        if cfg.diag == "gathers_only":
                            continue
                        oh = oh_p.tile([128, KT, 128], BF16, tag="oh")
                        nc.vector.tensor_tensor(
                            oh[:],
                            iota_b[:, 0:128].unsqueeze(1)
                                .broadcast_to([128, KT, 128]),
                            dst_t[g][:, ds(te * KT, KT)].unsqueeze(2)
                                .broadcast_to([128, KT, 128]),
                            mybir.AluOpType.is_equal)
                        rhs = oh_p.tile([128, KT, D], BF16, tag="gtb",
                                        name="gtb")
                        nc.vector.tensor_tensor(
                            rhs[:],
                            gt[:],
                            val_t[g][:, ds(te * KT, KT)].unsqueeze(2)
                                .broadcast_to([128, KT, D]),
                            mybir.AluOpType.mult)
                        ps = ps_p.tile([128, D], F32)
                        for c in range(KT):
                            nc.tensor.matmul(
                                ps[:], oh[:, c, :], rhs[:, c, :],
                                start=(c == 0), stop=(c == KT - 1),
                                skip_group_check=True)
                        nc.vector.scalar_tensor_tensor(
                            feat2d[:, ds(te * D, D)], ps[:],
                            wb_t[g][:, h:h + 1],
                            feat2d[:, ds(te * D, D)],
                            mybir.AluOpType.mult, mybir.AluOpType.add)
                        if h < HOP:
                            st = st_p.tile([128, D], F32)
                            nc.scalar.copy(st[:], ps[:])
                            nc.sync.dma_start(
                                cur_nxt[g][h][ds(te * 128, 128), :],
                                st[:])
                if h < HOP:
                    spread(h, g)

        # ---- write output: out[:, 0:D] = feat_s, out[:, D:2D] = feat_t ----
        for g, co in (("s", 0), ("t", D)):
            ob = once_p.tile([128, TILES, D], BF16, tag="ob", name=f"ob_{g}")
            nc.vector.tensor_copy(ob[:].rearrange("p t d -> p (t d)"),
                                  feat[g][:].rearrange("p t d -> p (t d)"))
            full_t = TILES - 1
            if full_t > 0:
                nc.sync.dma_start(
                    out_d[0:full_t * 128, co:co + D].rearrange(
                        "(t p) d -> p t d", p=128),
                    ob[:, 0:full_t, :])
            nc.sync.dma_start(
                out_d[full_t * 128:NPC, co:co + D],
                ob[0:TAIL, full_t, :])

    return nc


def _make_in_maps(cfg, inputs, arrs_s, arrs_t):
    import ml_dtypes
    x_s = np.asarray(inputs["x_s"], np.float32)
    x_t = np.asarray(inputs["x_t"], np.float32)
    w_s = np.asarray(inputs["w_s"], np.float32)
    w_t = np.asarray(inputs["w_t"], np.float32)
    wb_s = np.tile(w_s.reshape(1, -1), (128, 1)).astype(np.float32)
    wb_t = np.tile(w_t.reshape(1, -1), (128, 1)).astype(np.float32)
    iotab = np.tile(np.arange(128, dtype=np.float32), (128, 1))
    in_maps = []
    for c in range(cfg.CORES):
        xo_s = np.zeros((cfg.TILES * 128, cfg.D), ml_dtypes.bfloat16)
        xo_s[:cfg.NPC] = x_s[c * cfg.NPC:(c + 1) * cfg.NPC]
        xo_t = np.zeros((cfg.TILES * 128, cfg.D), ml_dtypes.bfloat16)
        xo_t[:cfg.NPC] = x_t[c * cfg.NPC:(c + 1) * cfg.NPC]
        im = {
            "xsh_s": xo_s, "xsh_t": xo_t,
            "idx_s": arrs_s[c]["idx"], "idx_t": arrs_t[c]["idx"],
            "val_s": arrs_s[c]["val"], "val_t": arrs_t[c]["val"],
            "dst_s": arrs_s[c]["dst"], "dst_t": arrs_t[c]["dst"],
            "wb_s": wb_s, "wb_t": wb_t,
            "iotab": iotab,
        }
        in_maps.append(im)
    return in_maps


def prepare(cfg, inputs):
    K_s, arrs_s = _preprocess_graph(
        cfg, inputs["A_rows"], inputs["A_cols"], inputs["A_vals"])
    K_t, arrs_t = _preprocess_graph(
        cfg, inputs["At_rows"], inputs["At_cols"], inputs["At_vals"])
    nc = build_program(cfg, K_s, K_t)
    nc.compile()
    in_maps = _make_in_maps(cfg, inputs, arrs_s, arrs_t)
    return nc, in_maps


def _kernel_overlapped(cfg, inputs) -> np.ndarray:
    """Custom PJRT runner: per-device input transfers are dispatched async
    BEFORE the Bass program is built/compiled, so the (slow) axon uploads
    stream in the background while the host works. Output buffers are
    zero-filled on device (nothing shipped), and the single bf16 output
    array is pulled and widened on the host."""
    import threading

    box = {}

    def _init_jax():
        import jax
        box["devices"] = jax.devices()[:cfg.CORES]

    th = threading.Thread(target=_init_jax)
    th.start()
    K_s, arrs_s = _preprocess_graph(
        cfg, inputs["A_rows"], inputs["A_cols"], inputs["A_vals"])
    K_t, arrs_t = _preprocess_graph(
        cfg, inputs["At_rows"], inputs["At_cols"], inputs["At_vals"])
    in_maps = _make_in_maps(cfg, inputs, arrs_s, arrs_t)
    th.join()

    import jax
    import jax.numpy as jnp
    from jax.sharding import Mesh, NamedSharding, PartitionSpec
    from jax.experimental.shard_map import shard_map
    from concourse import bass2jax
    from concourse.bass2jax import _bass_exec_p, partition_id_tensor

    devices = box["devices"]
    # Kick off all host->device shard transfers now (async under PJRT).
    futs = {n: [jax.device_put(in_maps[c][n], devices[c])
                for c in range(cfg.CORES)] for n in in_maps[0]}

    # Build + compile the Bass program while the uploads stream.
    nc = build_program(cfg, K_s, K_t)
    nc.compile()

    bass2jax.install_neuronx_cc_hook()
    partition_name = (nc.partition_id_tensor.name
                      if nc.partition_id_tensor else None)
    in_names, out_names, out_avals = [], [], []
    for alloc in nc.m.functions[0].allocations:
        if not isinstance(alloc, mybir.MemoryLocationSet):
            continue
        name = alloc.memorylocations[0].name
        if alloc.kind == "ExternalInput":
            if name != partition_name:
                in_names.append(name)
        elif alloc.kind == "ExternalOutput":
            out_names.append(name)
            out_avals.append(jax.core.ShapedArray(
                tuple(alloc.tensor_shape), mybir.dt.np(alloc.dtype)))
    n_params = len(in_names)
    n_outs = len(out_avals)
    all_names = list(in_names) + list(out_names)
    if partition_name is not None:
        all_names.append(partition_name)
    donate = tuple(range(n_params, n_params + n_outs))

    def _body(*args):
        operands = list(args)
        if partition_name is not None:
            operands.append(partition_id_tensor())
        outs = _bass_exec_p.bind(
            *operands, out_avals=tuple(out_avals),
            in_names=tuple(all_names), out_names=tuple(out_names),
            lowering_input_output_aliases=(), sim_require_finite=True,
            sim_require_nnan=True, nc=nc)
        return tuple(outs)

    mesh = Mesh(np.asarray(devices), ("core",))
    spec = PartitionSpec("core")
    sh = NamedSharding(mesh, spec)
    jitted = jax.jit(
        shard_map(_body, mesh=mesh, in_specs=(spec,) * (n_params + n_outs),
                  out_specs=(spec,) * n_outs, check_rep=False),
        donate_argnums=donate, keep_unused=True)

    gl = []
    for n in in_names:
        shards = futs[n]
        s0 = shards[0].shape
        gl.append(jax.make_array_from_single_device_arrays(
            (cfg.CORES * s0[0], *s0[1:]), sh, shards))
    zeros = []
    for av in out_avals:
        zf = jax.jit(
            lambda shape=av.shape, dt=av.dtype:
                jnp.zeros((cfg.CORES * shape[0], *shape[1:]), dt),
            out_shardings=sh)
        zeros.append(zf())

    outs = jitted(*gl, *zeros)
    out = np.asarray(outs[out_names.index("out")])
    return out.astype(np.float32)


def kernel(**inputs) -> np.ndarray:
    cfg = Cfg()
    try:
        return _kernel_overlapped(cfg, inputs)
    except Exception:
        nc, in_maps = prepare(cfg, inputs)
        res = run_bass_kernel_spmd(nc, in_maps, list(range(cfg.CORES)))
        return np.concatenate(
            [res.results[c]["out"].astype(np.float32)
             for c in range(cfg.CORES)], axis=0)
